# revision 24
# baseline (speedup 1.0000x reference)
"""Trainium2 Bass kernel for 8-head causal self-attention (b=2, s=4096, d=512, 8 heads x 64).

Sharding: 8 cores = 2 (batch) x 4 (head-pair). Core c handles batch c//4 and heads
(2*(c%4), 2*(c%4)+1). Each core computes a partial output projection over its two
heads' columns of W_O; the host sums the 4 partials per batch (tensor-parallel
all-reduce done on host at gather time).

Per-core algorithm ("everything transposed" layout, softmax over the partition axis):
  - per q-tile pipelined build: x^T via PE transposes (bf16), K^T/Q^T/V^T
    projections (2 heads packed), V tiles [p, 130] with fused ones-columns
  - S^T[p,q] blocks (128p x 512q) via row-tiled matmuls (2 heads concurrent),
    exp on ScalarE over double-buffered 2-bank PSUM groups (scale=1/8),
    causal diagonal masked post-exp with bf16 triangular masks on GpSimd
  - PV matmul accumulates z^T (+ sums row via the ones-column) in PSUM
  - per-partition reciprocal of sums (PE-transposed to [128,8]) and
    normalization fused into the per-head output-projection combine
"""

import numpy as np
import ml_dtypes
from contextlib import ExitStack

import concourse.bass as bass
import concourse.mybir as mybir
import concourse.tile as tile
from concourse import bacc
from concourse.bass import ts, ds
from concourse.masks import make_identity

BF16 = mybir.dt.bfloat16
F32 = mybir.dt.float32

B, S, D, NH, DH = 2, 4096, 512, 8, 64
N_CORES = 8
QT = 512          # q tile (free dim of S^T blocks)
PC = 128          # p chunk (partition dim of S^T blocks)


def build_attention_core(s=S, d=D, dh=DH):
    nqt = s // QT
    n_kc = d // 128
    nc = bacc.Bacc()
    x_dram = nc.dram_tensor("x", [s, d], F32, kind="ExternalInput")
    wkT_dram = nc.dram_tensor("wkT", [d, 2 * dh], BF16, kind="ExternalInput")
    wqT_dram = nc.dram_tensor("wqT", [d, 2 * dh], BF16, kind="ExternalInput")
    wvT_dram = nc.dram_tensor("wvT", [d, 2 * dh], BF16, kind="ExternalInput")
    woT_dram = nc.dram_tensor("woT", [2 * dh, d], BF16, kind="ExternalInput")
    out_dram = nc.dram_tensor("out", [s, d], F32, kind="ExternalOutput")

    with ExitStack() as ctx:
        tc = ctx.enter_context(tile.TileContext(nc))
        consts = ctx.enter_context(tc.tile_pool(name="consts", bufs=1))
        acts = ctx.enter_context(tc.tile_pool(name="acts", bufs=1))
        xstage = ctx.enter_context(tc.tile_pool(name="xstage", bufs=8))
        vstage = ctx.enter_context(tc.tile_pool(name="vstage", bufs=2))
        ptp = ctx.enter_context(tc.tile_pool(name="ptp", bufs=6))
        nrm = ctx.enter_context(tc.tile_pool(name="nrm", bufs=2))
        ost = ctx.enter_context(tc.tile_pool(name="ost", bufs=4))
        psp = ctx.enter_context(tc.tile_pool(name="psp", bufs=2, space="PSUM"))
        pzp = ctx.enter_context(tc.tile_pool(name="pzp", bufs=2, space="PSUM"))
        pmp = ctx.enter_context(tc.tile_pool(name="pmp", bufs=2, space="PSUM"))

        # ---- constants ----
        ident_f32 = consts.tile([128, 128], F32, tag="idf")
        make_identity(nc, ident_f32[:])
        ident_bf = consts.tile([128, 128], BF16, tag="idb")
        make_identity(nc, ident_bf[:])
        ones_row = consts.tile([1, dh], F32, tag="ones")
        nc.gpsimd.memset(ones_row[:], 1.0)
        diag_masks = []
        for j in range(QT // PC):
            mt = consts.tile([128, QT], BF16, tag=f"dgm{j}", name=f"dgm{j}")
            nc.gpsimd.memset(mt[:], 1.0)
            nc.gpsimd.affine_select(
                out=mt[:], in_=mt[:],
                compare_op=mybir.AluOpType.is_ge,
                fill=0.0, base=-PC * j,
                pattern=[[1, QT]], channel_multiplier=-1,
            )
            diag_masks.append(mt)

        # ---- persistent activations / weights ----
        xT = acts.tile([128, n_kc, s], BF16, tag="xT")
        kT = acts.tile([128, s], BF16, tag="kT")   # rows 0-63 head A, 64-127 head B
        qT = acts.tile([128, s], BF16, tag="qT")
        vtiles = acts.tile([128, s // PC, 2 * (dh + 1)], BF16, tag="vt")
        wk_sb = acts.tile([128, n_kc, 2 * dh], BF16, tag="wk")
        wq_sb = acts.tile([128, n_kc, 2 * dh], BF16, tag="wq")
        wv_sb = acts.tile([128, n_kc, 2 * dh], BF16, tag="wv")
        wo_sb = acts.tile([128, d], BF16, tag="wo")

        nc.sync.dma_start(wk_sb[:], wkT_dram.rearrange("(kc p) h -> p kc h", p=128))
        nc.sync.dma_start(wq_sb[:], wqT_dram.rearrange("(kc p) h -> p kc h", p=128))
        nc.sync.dma_start(wv_sb[:], wvT_dram.rearrange("(kc p) h -> p kc h", p=128))
        nc.sync.dma_start(wo_sb[:], woT_dram[:])
        ones_cols = vtiles[:, :, :].rearrange("p c (g hh) -> p c g hh", g=2)[:, :, :, dh : dh + 1]
        nc.gpsimd.memset(ones_cols, 1.0)

        def build(g):
            """Build x^T, K^T/Q^T/V^T and V tiles for p-slice [512g, 512g+512)."""
            xts = []
            for ti in range(4):
                xt = xstage.tile([128, d], F32, tag="xs", name=f"xs{g}_{ti}")
                nc.sync.dma_start(xt[:], x_dram[ts(4 * g + ti, 128), :])
                xb = xstage.tile([128, d], BF16, tag="xb", name=f"xb{g}_{ti}")
                nc.vector.tensor_copy(xb[:], xt[:])
                xts.append(xb)
            for half in range(2):
                ps2 = [
                    pmp.tile([128, 512], BF16, tag="pmisc", name=f"xtp{g}_{half}_{i}")
                    for i in range(2)
                ]
                for ti in range(4):
                    for i in range(2):
                        kc = 2 * half + i
                        nc.tensor.transpose(
                            ps2[i][:, ts(ti, 128)], xts[ti][:, ts(kc, 128)], ident_bf[:]
                        )
                for i in range(2):
                    kc = 2 * half + i
                    nc.vector.tensor_copy(xT[:, kc, ts(g, 512)], ps2[i][:])
            vts = vstage.tile([128, 512], BF16, tag="vts", name=f"vts{g}")
            for w_sb, dst in ((wk_sb, kT[:, ts(g, 512)]), (wq_sb, qT[:, ts(g, 512)]), (wv_sb, vts[:])):
                pj = pmp.tile([128, 512], F32, tag="pmisc", name=f"pj{g}")
                for kc in range(n_kc):
                    nc.tensor.matmul(
                        pj[:, :], w_sb[:, kc, :], xT[:, kc, ts(g, 512)],
                        start=(kc == 0), stop=(kc == n_kc - 1),
                    )
                nc.vector.tensor_copy(dst, pj[:, :])
            for i in range(4):
                pc = 4 * g + i
                vtp = pmp.tile([128, 128], BF16, tag="pmisc", name=f"vtp{g}_{i}")
                nc.tensor.transpose(vtp[:], vts[:, ts(i, 128)], ident_bf[:])
                dst = vtiles[:, pc, :].rearrange("p (gg hh) -> p gg hh", gg=2)[:, :, 0:dh]
                src = vtp[:].rearrange("p (gg hh) -> p gg hh", gg=2)
                nc.vector.tensor_copy(dst, src)

        def epi_rest(qt, zu, sums_sb):
            # deferred epilogue: reciprocal of sums + normalized output projection
            pms = pmp.tile([128, 512], F32, tag="pmisc", name=f"pms{qt}")
            for col in range(8):
                nc.tensor.transpose(
                    pms[:, col : col + 1], sums_sb[:, ts(col, 128)], ones_row[:, 0:1]
                )
            s128 = nrm.tile([128, 8], F32, tag="s128", name=f"s128_{qt}")
            nc.vector.tensor_copy(s128[:], pms[:, 0:8])
            rs128 = nrm.tile([128, 8], F32, tag="rs128", name=f"rs128_{qt}")
            nc.vector.reciprocal(rs128[:], s128[:])
            # rs128[:, 4h+qc] = 1/sum for head h at q = 128*qc + partition
            for qc in range(4):
                opsA = pmp.tile([128, 512], F32, tag="pmisc", name=f"opsA{qt}_{qc}")
                opsB = pmp.tile([128, 512], F32, tag="pmisc", name=f"opsB{qt}_{qc}")
                nc.tensor.matmul(
                    opsA[:, :], zu[0:dh, ts(qc, 128)], wo_sb[0:dh, :],
                    start=True, stop=True, tile_position=(0, 0),
                )
                nc.tensor.matmul(
                    opsB[:, :], zu[ds(dh, dh), ts(qc, 128)], wo_sb[ds(dh, dh), :],
                    start=True, stop=True, tile_position=(dh, 0),
                )
                osb = ost.tile([128, d], F32, tag="ob", name=f"ob{qt}_{qc}")
                nc.vector.tensor_scalar_mul(osb[:], opsA[:, :], rs128[:, qc : qc + 1])
                nc.vector.scalar_tensor_tensor(
                    osb[:], opsB[:, :], rs128[:, 4 + qc : 5 + qc], osb[:],
                    op0=mybir.AluOpType.mult, op1=mybir.AluOpType.add,
                )
                nc.sync.dma_start(out_dram[ds(QT * qt + 128 * qc, 128), :], osb[:])

        build(0)
        if nqt > 1:
            build(1)
        pending = None
        for qt in range(nqt):
            n_pc = (QT // PC) * (qt + 1)
            zps = [
                pzp.tile([dh + 1, 512], F32, tag="zps", name=f"zps{qt}_{h}")
                for h in range(2)
            ]
            for pc in range(n_pc):
                if pc == 2 and pending is not None:
                    epi_rest(*pending)
                    pending = None
                sps = psp.tile([128, 1024], F32, tag="sps", name=f"sps{qt}_{pc}")
                for h in range(2):
                    nc.tensor.matmul(
                        sps[:, ts(h, 512)],
                        kT[ds(dh * h, dh), ts(pc, 128)],
                        qT[ds(dh * h, dh), ts(qt, QT)],
                        start=True, stop=True,
                        tile_position=(dh * h, 0),
                    )
                pt_sb = ptp.tile([128, 1024], BF16, tag="pt", name=f"pt{qt}_{pc}")
                nc.scalar.activation(
                    pt_sb[:], sps[:], mybir.ActivationFunctionType.Exp,
                    bias=0.0, scale=1.0 / np.sqrt(dh).item(),
                )
                j = pc - (QT // PC) * qt
                if j >= 0:  # diagonal block: zero non-causal probs
                    for h in range(2):
                        nc.vector.tensor_mul(
                            pt_sb[:, ts(h, 512)], pt_sb[:, ts(h, 512)], diag_masks[j][:, :]
                        )
                for h in range(2):
                    nc.tensor.matmul(
                        zps[h][:, :],
                        vtiles[:, pc, ds((dh + 1) * h, dh + 1)],
                        pt_sb[:, ts(h, 512)],
                        start=(pc == 0), stop=(pc == n_pc - 1),
                    )

            # drain z^T + sums from PSUM immediately (frees pz slots)
            zu = nrm.tile([128, 512], BF16, tag="zu", name=f"zu{qt}")
            sums_sb = nrm.tile([1, 1024], F32, tag="sums", name=f"sums{qt}")
            for h in range(2):
                nc.vector.tensor_copy(zu[ds(dh * h, dh), :], zps[h][0:dh, :])
                nc.vector.tensor_copy(
                    sums_sb[:, ds(512 * h, 512)], zps[h][dh : dh + 1, :]
                )
            if qt + 2 < nqt:
                build(qt + 2)
            pending = (qt, zu, sums_sb)
        epi_rest(*pending)

    nc.finalize()
    return nc


_NC_CACHE = {}


def _get_nc(s=S):
    if s not in _NC_CACHE:
        _NC_CACHE[s] = build_attention_core(s=s)
    return _NC_CACHE[s]


def make_in_maps(x, W_K, W_Q, W_V, W_O):
    bf = ml_dtypes.bfloat16
    in_maps = []
    for c in range(N_CORES):
        b, hp = c // 4, c % 4
        hA, hB = 2 * hp, 2 * hp + 1
        wkT = np.concatenate([W_K[hA].T, W_K[hB].T], axis=1).astype(bf)  # [d, 128]
        wqT = np.concatenate([W_Q[hA].T, W_Q[hB].T], axis=1).astype(bf)
        wvT = np.concatenate([W_V[hA].T, W_V[hB].T], axis=1).astype(bf)
        woT = np.ascontiguousarray(W_O[:, DH * hA : DH * (hB + 1)].T).astype(bf)  # [128, d]
        in_maps.append(
            {
                "x": np.ascontiguousarray(x[b], dtype=np.float32),
                "wkT": np.ascontiguousarray(wkT),
                "wqT": np.ascontiguousarray(wqT),
                "wvT": np.ascontiguousarray(wvT),
                "woT": woT,
            }
        )
    return in_maps


def kernel(x, W_K, W_Q, W_V, W_O):
    from concourse.bass_utils import run_bass_kernel_spmd

    nc = _get_nc(S)
    in_maps = make_in_maps(x, W_K, W_Q, W_V, W_O)
    res = run_bass_kernel_spmd(nc, in_maps, core_ids=list(range(N_CORES)))
    out = np.zeros((B, S, D), dtype=np.float32)
    for c in range(N_CORES):
        out[c // 4] += res.results[c]["out"]
    return out
